# revision 13
# baseline (speedup 1.0000x reference)
"""Depthwise 5x5 conv (B=16, C=128, H=W=224, fp32) on 8 TRN2 NeuronCores.

Strategy
--------
Data-parallel over batch: each of the 8 cores handles 2 images.

On-core compute uses the TensorEngine via *banded matmuls*:
  out[c,i,j] = sum_{u,v} k[c,u,v] * x[c, i+u-2, j+v-2] + bias[c]

The vertical taps (u) are contracted by the PE array using a host-built
block-diagonal band matrix; the horizontal taps (v) become 5 column-shifted
matmuls accumulating into the same PSUM bank.

Partition layout (contraction dim K = 128):
  partition p = 32*b + r   : block b in 0..3 holds channel c = 4*g + b,
                             r = row offset within a 32-row input strip
  psum row  m = 28*b + mr  : output row mr (0..27) of block b's channel
lhsT[p, m] = k[4g+b, r-mr, v] when p//32 == m//28 == b and 0 <= r-mr <= 4.

The host pre-packs the input so each kernel iteration (one channel group g
of 4 channels x one 28-row output strip s, both images) reads a single
fully contiguous [128, 2*(W+8)] DRAM block: zero padding (2 conv-pad rows;
2 left + 6 right zero cols per image so all 5 column shifts stay in
bounds) is baked in, and with x data at padded column 2 the output column
j of image i lands exactly at psum column i*(W+8) + j. Results go to an
iteration-major DRAM scratch that the host reassembles.

Raw Bass (no Tile): the walrus build here only accepts ONE semaphore wait
per TPB instruction, so all waits are standalone EventSemaphore sequencer
instructions with hand-rolled ring-buffer semaphores. Pipeline rings are
4 deep; evictions (PSUM -> SBUF + bias) alternate DVE/ACT; slot-reuse is
serialized by the compute gates, so out-of-order DMA completion across
the 16 HW queues cannot repurpose a slot early.
"""

from contextlib import ExitStack

import numpy as np

import concourse.bass as bass
import concourse.mybir as mybir
from concourse.bass_utils import run_bass_kernel_spmd

F32 = mybir.dt.float32

# Problem geometry (hardcoded per spec nn_Conv_53798760350153)
B, C, H, W = 16, 128, 224, 224
KK, PAD = 5, 2
N_CORES = 8
BPC = B // N_CORES  # images per core = 2

# Tiling
CB = 4            # channels per 128-partition matmul (one per 32-row block)
RB = 32           # input rows per block (= M + 4)
M = RB - KK + 1   # output rows per strip per channel = 28
MBLK = CB * M     # psum partitions used = 112

NB = 4  # rhs ring depth
NP = 4  # psum ring depth
NO = 4  # output-tile ring depth


def build_program(c=C, h=H, w=W, bpc=BPC):
    """Build the per-core Bass program. All cores run the identical program
    on their own batch shard (pure data parallel, no collectives)."""
    wq = w + 2 * KK - 2    # padded cols per image (2 left + 6 right) = w + 8
    ng = c // CB           # channel groups
    ns = h // M            # row strips per image
    nit = ng * ns          # iterations
    nw = bpc * wq          # rhs/psum tile free width
    nmm = nw - KK + 1      # matmul free size (all 5 shifts stay in bounds)
    assert h % M == 0 and c % CB == 0
    assert nmm <= 512 and nw * 4 <= 2048, "psum tile must fit one bank"

    nc = bass.Bass()
    x_in = nc.declare_dram_parameter("x", [nit, 128, nw], F32, isOutput=False)
    w_in = nc.declare_dram_parameter("w", [128, ng * KK * MBLK], F32, isOutput=False)
    b_in = nc.declare_dram_parameter("b", [128, ng], F32, isOutput=False)
    out_t = nc.declare_dram_parameter("out", [nit, MBLK, bpc * w], F32, isOutput=True)

    with ExitStack() as ctx:
        ec = ctx.enter_context
        wt = ec(nc.sbuf_tensor("wt", [128, ng * KK * MBLK], F32))
        bt = ec(nc.sbuf_tensor("bt", [128, ng], F32))
        rhs = [ec(nc.sbuf_tensor(f"rhs{j}", [128, nw], F32))
               for j in range(NB)]
        ot = [ec(nc.sbuf_tensor(f"ot{j}", [MBLK, bpc * w], F32))
              for j in range(NO)]
        ps = [ec(nc.psum_tensor(f"ps{j}", [MBLK, nw], F32))
              for j in range(NP)]

        sem_w = ec(nc.semaphore("sem_w"))
        sem_in = [ec(nc.semaphore(f"sem_in{j}")) for j in range(NB)]
        sem_out = [ec(nc.semaphore(f"sem_out{j}")) for j in range(NO)]
        sem_pe = ec(nc.semaphore("sem_pe"))
        sem_ev = [ec(nc.semaphore("sem_evd")), ec(nc.semaphore("sem_eva"))]
        block = ec(nc.Block())

        def ev_count(k):
            """(sem, value) meaning 'eviction k is complete'."""
            return sem_ev[k % 2], k // 2 + 1

        @block.sync
        def _(sp):
            sp.dma_start(wt[:], w_in[:]).then_inc(sem_w, 16)
            sp.dma_start(bt[:], b_in[:]).then_inc(sem_w, 16)
            for k in range(nit + 2):
                if k < nit:
                    if k >= NB:
                        # rhs slot free once MM group k-NB has consumed it
                        sp.wait_ge(sem_pe, k - NB + 1)
                    sp.dma_start(rhs[k % NB][:], x_in[k]).then_inc(
                        sem_in[k % NB], 16)
                ko = k - 2
                if 0 <= ko < nit:
                    s, v = ev_count(ko)
                    sp.wait_ge(s, v)
                    sp.dma_start(out_t[ko], ot[ko % NO][:]).then_inc(
                        sem_out[ko % NO], 16)

        @block.tensor
        def _(pe):
            pe.wait_ge(sem_w, 32)
            for k in range(nit):
                g = k // ns
                pe.wait_ge(sem_in[k % NB], 16 * (k // NB + 1))
                if k >= NP:
                    s, v = ev_count(k - NP)
                    pe.wait_ge(s, v)
                p = ps[k % NP]
                for v in range(KK):
                    off = (g * KK + v) * MBLK
                    mm = nc.tensor.matmul(
                        p[:, 0:nmm],
                        wt[:, off:off + MBLK],
                        rhs[k % NB][:, v:v + nmm],
                        start=(v == 0),
                        stop=(v == KK - 1),
                    )
                mm.then_inc(sem_pe, 1)

        def evict(eng_wait, emit, parity):
            eng_wait.wait_ge(sem_w, 32)
            for k in range(parity, nit, 2):
                g = k // ns
                eng_wait.wait_ge(sem_pe, k + 1)
                if k >= NO:
                    # ot slot free once out-DMA k-NO completed
                    eng_wait.wait_ge(sem_out[k % NO], 16 * (k // NO))
                src = (ps[k % NP][:, :].rearrange("p (i q) -> p i q", i=bpc)
                       [:, :, 0:w])
                dst = ot[k % NO][:, :].rearrange("p (i q) -> p i q", i=bpc)
                emit(src, dst, g).then_inc(sem_ev[parity], 1)

        @block.vector
        def _(dve):
            evict(
                dve,
                lambda src, dst, g: nc.vector.tensor_scalar(
                    dst, src, bt[0:MBLK, g:g + 1], None, mybir.AluOpType.add),
                0,
            )

        @block.scalar
        def _(act):
            evict(
                act,
                lambda src, dst, g: nc.scalar.activation(
                    dst, src, mybir.ActivationFunctionType.Identity,
                    bias=bt[0:MBLK, g:g + 1]),
                1,
            )

    return nc


def host_prep(x, kern, bias_v, c=C, h=H, w=W):
    """Host-side packing: per-iteration contiguous input strips, band
    matrices, bias columns. x is the full batch [bsz, c, h, w]."""
    ng = c // CB
    ns = h // M
    hp, wq = h + 2 * PAD, w + 2 * KK - 2
    bsz = x.shape[0]

    x_pad = np.zeros((bsz, c, hp, wq), dtype=np.float32)
    x_pad[:, :, PAD:PAD + h, PAD:PAD + w] = x

    # xr[core, it=(g,s), p=(b,r), i, q] = x_pad[core*BPC+i, 4g+b, 28s+r, q]
    ncores = bsz // BPC
    xp = x_pad.reshape(ncores, BPC, ng, CB, hp, wq)
    xr = np.empty((ncores, ng, ns, CB, RB, BPC, wq), dtype=np.float32)
    for s in range(ns):
        # [ncores, BPC, ng, CB, RB, wq] -> [ncores, ng, CB, RB, BPC, wq]
        xr[:, :, s] = xp[:, :, :, :, M * s:M * s + RB, :].transpose(0, 2, 3, 4, 1, 5)
    xr = np.ascontiguousarray(xr.reshape(ncores, ng * ns, 128, BPC * wq))

    wd = np.zeros((128, ng * KK * MBLK), dtype=np.float32)
    mr = np.arange(M)
    for g in range(ng):
        for v in range(KK):
            col0 = (g * KK + v) * MBLK
            for b in range(CB):
                ch = CB * g + b
                for u in range(KK):
                    wd[RB * b + mr + u, col0 + M * b + mr] = kern[ch, u, v]

    bc = np.zeros((128, ng), dtype=np.float32)
    for g in range(ng):
        for b in range(CB):
            bc[M * b:M * (b + 1), g] = bias_v[CB * g + b]

    return xr, wd, bc


def host_post(raw, c=C, h=H, w=W):
    """Reassemble one core's [nit, MBLK, bpc*w] scratch into [bpc,c,h,w]."""
    ng, ns = c // CB, h // M
    r = raw.reshape(ng, ns, CB, M, BPC, w)
    return np.ascontiguousarray(
        r.transpose(4, 0, 2, 1, 3, 5).reshape(BPC, c, h, w))


_NC_CACHE = None


def kernel(**inputs):
    x = np.asarray(inputs["x"], dtype=np.float32)
    kern = np.asarray(inputs["kernel"], dtype=np.float32)
    bias_v = np.asarray(inputs["bias"], dtype=np.float32)

    xr, wd, bc = host_prep(x, kern, bias_v)

    global _NC_CACHE
    if _NC_CACHE is None:
        _NC_CACHE = build_program()
    nc = _NC_CACHE

    in_maps = [{"x": xr[i], "w": wd, "b": bc} for i in range(N_CORES)]
    res = run_bass_kernel_spmd(nc, in_maps, core_ids=list(range(N_CORES))).results
    return np.concatenate([host_post(r["out"]) for r in res], axis=0)


# revision 16
# speedup vs baseline: 1.0455x; 1.0455x over previous
"""Depthwise 5x5 conv (B=16, C=128, H=W=224, fp32) on 8 TRN2 NeuronCores.

Strategy
--------
Data-parallel over batch: each of the 8 cores handles 2 images.

On-core compute uses the TensorEngine via *banded matmuls*:
  out[c,i,j] = sum_{u,v} k[c,u,v] * x[c, i+u-2, j+v-2] + bias[c]

The vertical taps (u) are contracted by the PE array using a host-built
block-diagonal band matrix; the horizontal taps (v) become 5 column-shifted
matmuls accumulating into the same PSUM bank.

Partition layout (contraction dim K = 128):
  partition p = 32*b + r   : block b in 0..3 holds channel c = 4*g + b,
                             r = row offset within a 32-row input strip
  psum row  m = 28*b + mr  : output row mr (0..27) of block b's channel
lhsT[p, m] = k[4g+b, r-mr, v] when p//32 == m//28 == b and 0 <= r-mr <= 4.

The host pre-packs the input so each kernel iteration (one channel group g
of 4 channels x one 28-row output strip s, both images) reads a single
fully contiguous [128, 2*(W+8)] DRAM block: zero padding (2 conv-pad rows;
2 left + 6 right zero cols per image so all 5 column shifts stay in
bounds) is baked in, and with x data at padded column 2 the output column
j of image i lands exactly at psum column i*(W+8) + j. Results go to an
iteration-major DRAM scratch that the host reassembles.

Raw Bass (no Tile): the walrus build here only accepts ONE semaphore wait
per TPB instruction, so all waits are standalone EventSemaphore sequencer
instructions with hand-rolled ring-buffer semaphores. Pipeline rings are
4 deep; evictions (PSUM -> SBUF + bias) alternate DVE/ACT; slot-reuse is
serialized by the compute gates, so out-of-order DMA completion across
the 16 HW queues cannot repurpose a slot early.
"""

from contextlib import ExitStack

import numpy as np

import concourse.bass as bass
import concourse.mybir as mybir
from concourse.bass_utils import run_bass_kernel_spmd

F32 = mybir.dt.float32

# Problem geometry (hardcoded per spec nn_Conv_53798760350153)
B, C, H, W = 16, 128, 224, 224
KK, PAD = 5, 2
N_CORES = 8
BPC = B // N_CORES  # images per core = 2

# Tiling
CB = 4            # channels per 128-partition matmul (one per 32-row block)
RB = 32           # input rows per block (= M + 4)
M = RB - KK + 1   # output rows per strip per channel = 28
MBLK = CB * M     # psum partitions used = 112

NB = 4  # rhs ring depth
NP = 4  # psum ring depth
NO = 4  # output-tile ring depth


def build_program(c=C, h=H, w=W, bpc=BPC, max_it=None):
    """Build the per-core Bass program. All cores run the identical program
    on their own batch shard (pure data parallel, no collectives).
    max_it truncates the iteration count (benchmarking only)."""
    wq = w + 2 * KK - 2    # padded cols per image (2 left + 6 right) = w + 8
    ng = c // CB           # channel groups
    ns = h // M            # row strips per image
    nit = ng * ns          # iterations
    if max_it is not None:
        nit = min(nit, max_it)
    nw = bpc * wq          # rhs/psum tile free width
    nmm = nw - KK + 1      # matmul free size (all 5 shifts stay in bounds)
    assert h % M == 0 and c % CB == 0
    assert nmm <= 512 and nw * 4 <= 2048, "psum tile must fit one bank"

    nc = bass.Bass()
    x_in = nc.declare_dram_parameter("x", [ng * ns, 128, nw], F32, isOutput=False)
    w_in = nc.declare_dram_parameter("w", [128, ng * KK * MBLK], F32, isOutput=False)
    b_in = nc.declare_dram_parameter("b", [128, ng], F32, isOutput=False)
    out_t = nc.declare_dram_parameter("out", [ng * ns, MBLK, bpc * w], F32,
                                      isOutput=True)

    with ExitStack() as ctx:
        ec = ctx.enter_context
        wt = ec(nc.sbuf_tensor("wt", [128, ng * KK * MBLK], F32))
        bt = ec(nc.sbuf_tensor("bt", [128, ng], F32))
        rhs = [ec(nc.sbuf_tensor(f"rhs{j}", [128, nw], F32))
               for j in range(NB)]
        ot = [ec(nc.sbuf_tensor(f"ot{j}", [MBLK, bpc * w], F32))
              for j in range(NO)]
        ps = [ec(nc.psum_tensor(f"ps{j}", [MBLK, nw], F32))
              for j in range(NP)]

        sem_w = ec(nc.semaphore("sem_w"))
        sem_in = [ec(nc.semaphore(f"sem_in{j}")) for j in range(NB)]
        sem_out = [ec(nc.semaphore(f"sem_out{j}")) for j in range(NO)]
        sem_pe = ec(nc.semaphore("sem_pe"))
        sem_ev = [ec(nc.semaphore("sem_evd")), ec(nc.semaphore("sem_eva"))]
        block = ec(nc.Block())

        def ev_count(k):
            """(sem, value) meaning 'eviction k is complete'."""
            return sem_ev[k % 2], k // 2 + 1

        @block.sync
        def _(sp):
            sp.dma_start(wt[:], w_in[:]).then_inc(sem_w, 16)
            sp.dma_start(bt[:], b_in[:]).then_inc(sem_w, 16)
            for k in range(nit + 2):
                if k < nit:
                    if k >= NB:
                        # rhs slot free once MM group k-NB has consumed it
                        sp.wait_ge(sem_pe, k - NB + 1)
                    sp.dma_start(rhs[k % NB][:], x_in[k]).then_inc(
                        sem_in[k % NB], 16)
                ko = k - 2
                if 0 <= ko < nit:
                    s, v = ev_count(ko)
                    sp.wait_ge(s, v)
                    sp.dma_start(out_t[ko], ot[ko % NO][:]).then_inc(
                        sem_out[ko % NO], 16)

        @block.tensor
        def _(pe):
            pe.wait_ge(sem_w, 32)
            for k in range(nit):
                g = k // ns
                pe.wait_ge(sem_in[k % NB], 16 * (k // NB + 1))
                if k >= NP:
                    s, v = ev_count(k - NP)
                    pe.wait_ge(s, v)
                p = ps[k % NP]
                for v in range(KK):
                    off = (g * KK + v) * MBLK
                    mm = nc.tensor.matmul(
                        p[:, 0:nmm],
                        wt[:, off:off + MBLK],
                        rhs[k % NB][:, v:v + nmm],
                        start=(v == 0),
                        stop=(v == KK - 1),
                    )
                mm.then_inc(sem_pe, 1)

        def evict(eng_wait, emit, parity):
            eng_wait.wait_ge(sem_w, 32)
            for k in range(parity, nit, 2):
                g = k // ns
                eng_wait.wait_ge(sem_pe, k + 1)
                if k >= NO:
                    # ot slot free once out-DMA k-NO completed
                    eng_wait.wait_ge(sem_out[k % NO], 16 * (k // NO))
                src = (ps[k % NP][:, :].rearrange("p (i q) -> p i q", i=bpc)
                       [:, :, 0:w])
                dst = ot[k % NO][:, :].rearrange("p (i q) -> p i q", i=bpc)
                emit(src, dst, g).then_inc(sem_ev[parity], 1)

        @block.vector
        def _(dve):
            evict(
                dve,
                lambda src, dst, g: nc.vector.tensor_scalar(
                    dst, src, bt[0:MBLK, g:g + 1], None, mybir.AluOpType.add),
                0,
            )

        @block.scalar
        def _(act):
            evict(
                act,
                lambda src, dst, g: nc.scalar.activation(
                    dst, src, mybir.ActivationFunctionType.Identity,
                    bias=bt[0:MBLK, g:g + 1]),
                1,
            )

    return nc


def host_prep(x, kern, bias_v, c=C, h=H, w=W):
    """Host-side packing: per-iteration contiguous input strips, band
    matrices, bias columns. x is the full batch [bsz, c, h, w]."""
    ng = c // CB
    ns = h // M
    hp, wq = h + 2 * PAD, w + 2 * KK - 2
    bsz = x.shape[0]

    x_pad = np.zeros((bsz, c, hp, wq), dtype=np.float32)
    x_pad[:, :, PAD:PAD + h, PAD:PAD + w] = x

    # xr[core, it=(g,s), p=(b,r), i, q] = x_pad[core*BPC+i, 4g+b, 28s+r, q]
    ncores = bsz // BPC
    xp = x_pad.reshape(ncores, BPC, ng, CB, hp, wq)
    xr = np.empty((ncores, ng, ns, CB, RB, BPC, wq), dtype=np.float32)
    for s in range(ns):
        # [ncores, BPC, ng, CB, RB, wq] -> [ncores, ng, CB, RB, BPC, wq]
        xr[:, :, s] = xp[:, :, :, :, M * s:M * s + RB, :].transpose(0, 2, 3, 4, 1, 5)
    xr = np.ascontiguousarray(xr.reshape(ncores, ng * ns, 128, BPC * wq))

    wd = np.zeros((128, ng * KK * MBLK), dtype=np.float32)
    mr = np.arange(M)
    for g in range(ng):
        for v in range(KK):
            col0 = (g * KK + v) * MBLK
            for b in range(CB):
                ch = CB * g + b
                for u in range(KK):
                    wd[RB * b + mr + u, col0 + M * b + mr] = kern[ch, u, v]

    bc = np.zeros((128, ng), dtype=np.float32)
    for g in range(ng):
        for b in range(CB):
            bc[M * b:M * (b + 1), g] = bias_v[CB * g + b]

    return xr, wd, bc


def host_post(raw, c=C, h=H, w=W):
    """Reassemble one core's [nit, MBLK, bpc*w] scratch into [bpc,c,h,w]."""
    ng, ns = c // CB, h // M
    r = raw.reshape(ng, ns, CB, M, BPC, w)
    return np.ascontiguousarray(
        r.transpose(4, 0, 2, 1, 3, 5).reshape(BPC, c, h, w))


_NC_CACHE = None


def kernel(**inputs):
    x = np.asarray(inputs["x"], dtype=np.float32)
    kern = np.asarray(inputs["kernel"], dtype=np.float32)
    bias_v = np.asarray(inputs["bias"], dtype=np.float32)

    xr, wd, bc = host_prep(x, kern, bias_v)

    global _NC_CACHE
    if _NC_CACHE is None:
        _NC_CACHE = build_program()
    nc = _NC_CACHE

    in_maps = [{"x": xr[i], "w": wd, "b": bc} for i in range(N_CORES)]
    res = run_bass_kernel_spmd(nc, in_maps, core_ids=list(range(N_CORES))).results
    return np.concatenate([host_post(r["out"]) for r in res], axis=0)


# revision 19
# speedup vs baseline: 1.4096x; 1.3483x over previous
"""Depthwise 5x5 conv (B=16, C=128, H=W=224, fp32) on 8 TRN2 NeuronCores.

Strategy
--------
Data-parallel over batch: each of the 8 cores handles 2 images.

On-core compute uses the TensorEngine via *banded matmuls*:
  out[c,i,j] = sum_{u,v} k[c,u,v] * x[c, i+u-2, j+v-2] + bias[c]

The vertical taps (u) are contracted by the PE array using a host-built
block-diagonal band matrix; the horizontal taps (v) become 5 column-shifted
matmuls accumulating into the same PSUM bank.

Partition layout (contraction dim K = 128):
  partition p = 32*b + r   : block b in 0..3 holds channel c = 4*g + b,
                             r = row offset within a 32-row input strip
  psum row  m = 28*b + mr  : output row mr (0..27) of block b's channel
lhsT[p, m] = k[4g+b, r-mr, v] when p//32 == m//28 == b and 0 <= r-mr <= 4.

The host pre-packs the input so each kernel iteration (one channel group g
of 4 channels x one 28-row output strip s, both images) reads a single
fully contiguous [128, 2*(W+8)] DRAM block: zero padding (2 conv-pad rows;
2 left + 6 right zero cols per image so all 5 column shifts stay in
bounds) is baked in, and with x data at padded column 2 the output column
j of image i lands exactly at psum column i*(W+8) + j. Results go to an
iteration-major DRAM scratch that the host reassembles.

Raw Bass (no Tile): the walrus build here only accepts ONE semaphore wait
per TPB instruction, so all waits are standalone EventSemaphore sequencer
instructions with hand-rolled ring-buffer semaphores. Pipeline rings are
4 deep; evictions (PSUM -> SBUF + bias) alternate DVE/ACT; slot-reuse is
serialized by the compute gates, so out-of-order DMA completion across
the 16 HW queues cannot repurpose a slot early.
"""

from contextlib import ExitStack

import numpy as np

import concourse.bass as bass
import concourse.mybir as mybir
from concourse.bass_utils import run_bass_kernel_spmd

F32 = mybir.dt.float32

# Problem geometry (hardcoded per spec nn_Conv_53798760350153)
B, C, H, W = 16, 128, 224, 224
KK, PAD = 5, 2
N_CORES = 8
BPC = B // N_CORES  # images per core = 2

# Tiling
CB = 4            # channels per 128-partition matmul (one per 32-row block)
RB = 32           # input rows per block (= M + 4)
M = RB - KK + 1   # output rows per strip per channel = 28
MBLK = CB * M     # psum partitions used = 112

NB = 4  # rhs ring depth
NP = 4  # psum ring depth
NO = 4  # output-tile ring depth


def build_program(c=C, h=H, w=W, bpc=BPC, max_it=None, variant="full"):
    """Build the per-core Bass program. All cores run the identical program
    on their own batch shard (pure data parallel, no collectives).
    max_it truncates the iteration count; variant in {"full", "dmaonly",
    "noout"} strips stages (both benchmarking only)."""
    wq = w + 2 * KK - 2    # padded cols per image (2 left + 6 right) = w + 8
    ng = c // CB           # channel groups
    ns = h // M            # row strips per image
    nit = ng * ns          # iterations
    if max_it is not None:
        nit = min(nit, max_it)
    nw = bpc * wq          # rhs/psum tile free width
    nmm = nw - KK + 1      # matmul free size (all 5 shifts stay in bounds)
    assert h % M == 0 and c % CB == 0
    assert nmm <= 512 and nw * 4 <= 2048, "psum tile must fit one bank"

    nc = bass.Bass()
    x_in = nc.declare_dram_parameter("x", [ng * ns, 128, nw], F32, isOutput=False)
    w_in = nc.declare_dram_parameter("w", [128, ng * KK * MBLK], F32, isOutput=False)
    b_in = nc.declare_dram_parameter("b", [128, ng], F32, isOutput=False)
    out_t = nc.declare_dram_parameter("out", [ng * ns, MBLK, bpc * w], F32,
                                      isOutput=True)

    with ExitStack() as ctx:
        ec = ctx.enter_context
        wt = ec(nc.sbuf_tensor("wt", [128, ng * KK * MBLK], F32))
        bt = ec(nc.sbuf_tensor("bt", [128, ng], F32))
        rhs = [ec(nc.sbuf_tensor(f"rhs{j}", [128, nw], F32))
               for j in range(NB)]
        ot = [ec(nc.sbuf_tensor(f"ot{j}", [MBLK, bpc * w], F32))
              for j in range(NO)]
        ps = [ec(nc.psum_tensor(f"ps{j}", [MBLK, nw], F32))
              for j in range(NP)]

        sem_w = ec(nc.semaphore("sem_w"))
        sem_in = [ec(nc.semaphore(f"sem_in{j}")) for j in range(NB)]
        sem_out = [ec(nc.semaphore(f"sem_out{j}")) for j in range(NO)]
        sem_pe = ec(nc.semaphore("sem_pe"))
        sem_ev = [ec(nc.semaphore("sem_evd")), ec(nc.semaphore("sem_eva"))]
        block = ec(nc.Block())

        def ev_count(k):
            """(sem, value) meaning 'eviction k is complete'."""
            return sem_ev[k % 2], k // 2 + 1

        if variant == "dmaonly":
            # Timing probe: pure DMA streams, no compute, no sync.
            @block.sync
            def _(sp):
                for k in range(nit):
                    sp.dma_start(rhs[k % NB][:], x_in[k]).then_inc(
                        sem_in[k % NB], 16)
                    sp.dma_start(out_t[k], ot[k % NO][:]).then_inc(
                        sem_out[k % NO], 16)
                for j in range(NB):
                    sp.wait_ge(sem_in[j], 16 * (nit // NB))
                for j in range(NO):
                    sp.wait_ge(sem_out[j], 16 * (nit // NO))
            return nc

        @block.sync
        def _(sp):
            sp.dma_start(wt[:], w_in[:]).then_inc(sem_w, 16)
            sp.dma_start(bt[:], b_in[:]).then_inc(sem_w, 16)
            for k in range(nit + 2):
                if k < nit:
                    if k >= NB:
                        # rhs slot free once MM group k-NB has consumed it
                        sp.wait_ge(sem_pe, k - NB + 1)
                    sp.dma_start(rhs[k % NB][:], x_in[k]).then_inc(
                        sem_in[k % NB], 16)
                ko = k - 2
                if variant != "noout" and 0 <= ko < nit:
                    s, v = ev_count(ko)
                    sp.wait_ge(s, v)
                    sp.dma_start(out_t[ko], ot[ko % NO][:]).then_inc(
                        sem_out[ko % NO], 16)

        @block.tensor
        def _(pe):
            pe.wait_ge(sem_w, 32)
            for k in range(nit):
                g = k // ns
                pe.wait_ge(sem_in[k % NB], 16 * (k // NB + 1))
                if k >= NP:
                    s, v = ev_count(k - NP)
                    pe.wait_ge(s, v)
                p = ps[k % NP]
                for v in range(KK):
                    off = (g * KK + v) * MBLK
                    mm = nc.tensor.matmul(
                        p[:, 0:nmm],
                        wt[:, off:off + MBLK],
                        rhs[k % NB][:, v:v + nmm],
                        start=(v == 0),
                        stop=(v == KK - 1),
                    )
                mm.then_inc(sem_pe, 1)

        def evict(eng_wait, emit, parity):
            eng_wait.wait_ge(sem_w, 32)
            for k in range(parity, nit, 2):
                g = k // ns
                eng_wait.wait_ge(sem_pe, k + 1)
                if variant != "noout" and k >= NO:
                    # ot slot free once out-DMA k-NO completed
                    eng_wait.wait_ge(sem_out[k % NO], 16 * (k // NO))
                src = (ps[k % NP][:, :].rearrange("p (i q) -> p i q", i=bpc)
                       [:, :, 0:w])
                dst = ot[k % NO][:, :].rearrange("p (i q) -> p i q", i=bpc)
                emit(src, dst, g).then_inc(sem_ev[parity], 1)

        @block.vector
        def _(dve):
            evict(
                dve,
                lambda src, dst, g: nc.vector.tensor_scalar(
                    dst, src, bt[0:MBLK, g:g + 1], None, mybir.AluOpType.add),
                0,
            )

        @block.scalar
        def _(act):
            evict(
                act,
                lambda src, dst, g: nc.scalar.activation(
                    dst, src, mybir.ActivationFunctionType.Identity,
                    bias=bt[0:MBLK, g:g + 1]),
                1,
            )

    return nc


def host_prep(x, kern, bias_v, c=C, h=H, w=W):
    """Host-side packing: per-iteration contiguous input strips, band
    matrices, bias columns. x is the full batch [bsz, c, h, w]."""
    ng = c // CB
    ns = h // M
    hp, wq = h + 2 * PAD, w + 2 * KK - 2
    bsz = x.shape[0]

    x_pad = np.zeros((bsz, c, hp, wq), dtype=np.float32)
    x_pad[:, :, PAD:PAD + h, PAD:PAD + w] = x

    # xr[core, it=(g,s), p=(b,r), i, q] = x_pad[core*BPC+i, 4g+b, 28s+r, q]
    ncores = bsz // BPC
    xp = x_pad.reshape(ncores, BPC, ng, CB, hp, wq)
    xr = np.empty((ncores, ng, ns, CB, RB, BPC, wq), dtype=np.float32)
    for s in range(ns):
        # [ncores, BPC, ng, CB, RB, wq] -> [ncores, ng, CB, RB, BPC, wq]
        xr[:, :, s] = xp[:, :, :, :, M * s:M * s + RB, :].transpose(0, 2, 3, 4, 1, 5)
    xr = np.ascontiguousarray(xr.reshape(ncores, ng * ns, 128, BPC * wq))

    wd = np.zeros((128, ng * KK * MBLK), dtype=np.float32)
    mr = np.arange(M)
    for g in range(ng):
        for v in range(KK):
            col0 = (g * KK + v) * MBLK
            for b in range(CB):
                ch = CB * g + b
                for u in range(KK):
                    wd[RB * b + mr + u, col0 + M * b + mr] = kern[ch, u, v]

    bc = np.zeros((128, ng), dtype=np.float32)
    for g in range(ng):
        for b in range(CB):
            bc[M * b:M * (b + 1), g] = bias_v[CB * g + b]

    return xr, wd, bc


def host_post(raw, c=C, h=H, w=W):
    """Reassemble one core's [nit, MBLK, bpc*w] scratch into [bpc,c,h,w]."""
    ng, ns = c // CB, h // M
    r = raw.reshape(ng, ns, CB, M, BPC, w)
    return np.ascontiguousarray(
        r.transpose(4, 0, 2, 1, 3, 5).reshape(BPC, c, h, w))


_NC_CACHE = None


def kernel(**inputs):
    x = np.asarray(inputs["x"], dtype=np.float32)
    kern = np.asarray(inputs["kernel"], dtype=np.float32)
    bias_v = np.asarray(inputs["bias"], dtype=np.float32)

    xr, wd, bc = host_prep(x, kern, bias_v)

    global _NC_CACHE
    if _NC_CACHE is None:
        _NC_CACHE = build_program()
    nc = _NC_CACHE

    in_maps = [{"x": xr[i], "w": wd, "b": bc} for i in range(N_CORES)]
    res = run_bass_kernel_spmd(nc, in_maps, core_ids=list(range(N_CORES))).results
    return np.concatenate([host_post(r["out"]) for r in res], axis=0)
